# revision 6
# baseline (speedup 1.0000x reference)
"""Distributed Trainium2 kernel for a single attention head.

Problem: x:[8,2048,1024] f32, w_q/w_k/w_v:[1024,64] f32
  q,k,v = x@w ; scores = (q k^T)/sqrt(1024) causal-masked; out = softmax(scores)@v

Sharding: data-parallel over batch B=8 across the 8 NeuronCores (one batch
element per core, weights replicated, no collectives).

Per-core dataflow (T=2048, C=1024, H=64). v2 of the kernel; the structural
changes vs v1 are geared at PE-stream density and DVE offload:

  - blob0 = [wqk | x^T chunk0 | wv] so the first DMA cut carries exactly
    what the first projection matmuls need; xrest chunks ship in half-chunk
    cuts for finer arrival granularity. All input, dup and output DMAs ride
    the sync HWDGE queue in FIFO order (SWDGE latency ~4us is too slow for
    anything on the critical path); gpsimd SWDGE only carries mask/identity
    constants.
  - qk projection epilogue: ONE [128,512] DVE cast into qk_sb (q rows 0:64,
    k rows 64:128 as produced by the packed weights), then two sync-queue
    dup DMAs build dup_t (k at rows 0:64, q at rows 64:128). Scores for
    s-block j with moving q of chunk t: half-0 MM reads k from dup_t/q from
    qk_sb; half-1 MM reads k from qk_sb/q from dup_t. All pairs of chunks
    1-3 run row-tiled 2x concurrent; chunk-0 pairs run unpacked on half 1
    (q-dup only is on the critical path there).
  - v projections: both chunks of a pair column-packed into ONE [128,512]
    PSUM tile -> ONE cast to vTab (ca rows 0:64, cb rows 64:128) -> 8
    combined [128,128] PE transposes produce v1 data for BOTH chunks at
    once -> one strided DVE copy per transpose into a [128,130] v1-pair
    tile ([v|1] slots for both chunks); ones column memset on gpsimd.
  - exp on ScalarE trimmed to [lo0:1024] per pair; the last pair (3,14)
    is split per block so the tail epilogue starts ~0.6us earlier.
  - epilogue per half-chunk (cols 0:256 after the diagonal pair, 256:512
    after the next): PSUM->SBUF copy, 2 transposes into one [128,2,65]
    PSUM tile, ONE [128,2] reciprocal, broadcast multiply, DMA out.
  - causal: diagonal 128x128 blocks multiplied by a 0/1 mask on VectorE.
  - PV unchanged: out^T accumulated with lhsT = [v | 1] fused denominator,
    LAG slots behind scores.
"""

import os
import sys

import numpy as np

for p in ("/opt/trn_rl_repo",):
    if p not in sys.path and os.path.isdir(p):
        sys.path.insert(0, p)

import ml_dtypes  # noqa: E402

B, T, C, H = 8, 2048, 1024, 64
N_CORES = 8
TCH = 512                  # t-chunk
N_CHUNK = T // TCH         # 4
N_CT = C // 128            # 8 contraction tiles
SCALE = float(C) ** -0.5   # 1/32
N_WARM = 7                 # PE warm-up matmuls (N=512 each, ~0.43us cold)
LAG = 4                    # PV trails scores by this many pair slots
FUSED_EPI = True           # broadcast-mul epilogue (fallback: per-block)

_CACHE = {}


def _build():
    """Build + compile the SPMD Bass graph (same graph on all 8 cores)."""
    import concourse.bass as bass
    import concourse.mybir as mybir
    import concourse.tile as tile
    from concourse import bacc

    f32 = mybir.dt.float32
    bf16 = mybir.dt.bfloat16
    EXP = mybir.ActivationFunctionType.Exp

    nc = bacc.Bacc(
        "TRN2", target_bir_lowering=False, debug=False, num_devices=N_CORES
    )

    # blob0 = [wqk | x^T chunk0 | wv], x pre-tiled [c-tile][t]
    W_WQK = N_CT * 128
    W_X = N_CT * TCH
    W_WV = N_CT * H
    BLOB0_W = W_WQK + W_X + W_WV
    blob0_d = nc.dram_tensor("blob0", [128, BLOB0_W], bf16, kind="ExternalInput")
    xrest_d = nc.dram_tensor("xrest", [128, 3 * W_X], bf16, kind="ExternalInput")
    mask_d = nc.dram_tensor("mask01", [128, 128], bf16, kind="ExternalInput")
    idf_d = nc.dram_tensor("idf", [128, 128], f32, kind="ExternalInput")
    idb_d = nc.dram_tensor("idb", [128, 128], bf16, kind="ExternalInput")
    out_d = nc.dram_tensor("out", [T, H], f32, kind="ExternalOutput")

    with tile.TileContext(nc) as tc:
        with (
            tc.tile_pool(name="const", bufs=1) as constp,
            tc.tile_pool(name="xTp", bufs=1) as xTp,
            tc.tile_pool(name="qkp", bufs=1) as qkp,
            tc.tile_pool(name="vTp", bufs=2) as vTp,
            tc.tile_pool(name="v1p", bufs=1) as v1p,
            tc.tile_pool(name="exp", bufs=LAG + 2) as expp,
            tc.tile_pool(name="epi", bufs=2) as epip,
            tc.tile_pool(name="Sp", bufs=2, space="PSUM") as Sp,
            tc.tile_pool(name="accp", bufs=1, space="PSUM") as accp,
            tc.tile_pool(name="miscp", bufs=3, space="PSUM") as miscp,
        ):
            # ---- PE warm-up scratch (memset on gpsimd: earliest free engine)
            warm_sb = constp.tile([128, TCH], bf16, tag="warm_sb", name="warm_sb")
            nc.gpsimd.memset(warm_sb[:], 0.0)
            warm_act = constp.tile([128, 8], bf16, tag="warm_act", name="warm_act")
            warm_ps = miscp.tile([128, TCH], f32, tag="misc", name="warm_ps")
            for i in range(N_WARM):
                nc.tensor.matmul(
                    warm_ps[:, :],
                    warm_sb[:, 0:128],
                    warm_sb[:, :],
                    start=True,
                    stop=True,
                    skip_group_check=True,
                )

            # ---- input DMAs: all on the sync HWDGE queue, strict FIFO ----
            blob0_t = constp.tile([128, BLOB0_W], bf16, tag="blob0", name="blob0_t")
            # cuts: [wqk|t0-1], [t2-4], [t5-7], [wv]. Few, large DMAs: many
            # small cuts measurably degrade ring throughput (~20%) and the
            # per-engine completion spread (sem fires on the slowest of 16
            # engines) grows to ~3us.
            cuts = [
                0,
                W_WQK + 2 * TCH,
                W_WQK + 5 * TCH,
                W_WQK + W_X,
                BLOB0_W,
            ]
            for lo, hi in zip(cuts[:-1], cuts[1:]):
                nc.sync.dma_start(out=blob0_t[:, lo:hi], in_=blob0_d[:, lo:hi])
            wqk_t = blob0_t[:, 0:W_WQK].rearrange("p (n m) -> p n m", n=N_CT)
            xt = {
                0: blob0_t[:, W_WQK : W_WQK + W_X].rearrange(
                    "p (n m) -> p n m", n=N_CT
                )
            }
            wv_t = blob0_t[:, W_WQK + W_X :].rearrange("p (n m) -> p n m", n=N_CT)
            # xrest: chunk 1 in halves (its projection is tight behind the
            # front), chunks 2-3 whole
            xr_h = xrest_d[:].rearrange("p (t h n m) -> p t h n m", t=3, h=2, n=N_CT // 2)
            xr_f = xrest_d[:].rearrange("p (t n m) -> p t n m", t=3, n=N_CT)
            for t in (1, 2, 3):
                xx = xTp.tile([128, N_CT, TCH], bf16, tag=f"x{t}", name=f"x{t}")
                if t == 1:
                    xxh = xx[:].rearrange("p (h n) m -> p h n m", h=2)
                    for hh in range(2):
                        nc.sync.dma_start(out=xxh[:, hh], in_=xr_h[:, t - 1, hh])
                else:
                    nc.sync.dma_start(out=xx[:], in_=xr_f[:, t - 1])
                xt[t] = xx[:]

            # dummy exp: forces the ACT table-set load during the DMA phase
            nc.scalar.activation(warm_act[:], warm_sb[:, 0:8], EXP, scale=1.0)

            # ---- small constants on the gpsimd SWDGE queue ----
            mask_t = constp.tile([128, 128], bf16, tag="mask", name="mask_t")
            nc.gpsimd.dma_start(out=mask_t[:], in_=mask_d[:])
            idb_t = constp.tile([128, 128], bf16, tag="idb", name="idb_t")
            nc.gpsimd.dma_start(out=idb_t[:], in_=idb_d[:])
            idf_t = constp.tile([128, 128], f32, tag="idf", name="idf_t")
            nc.gpsimd.dma_start(out=idf_t[:], in_=idf_d[:])

            # per-chunk q/k tiles: qh[half] / kh[half] -> tile holding that
            # operand on that partition half (qk_sb or dup_t)
            qh = {}
            kh = {}
            v1 = {}    # [128, 65] bf16 AP per s-tile: [v | 1]

            def qk_steps(tch):
                """Emission thunks for chunk `tch`'s q/k projection."""
                steps = []
                state = {}

                def qk_mm(c):
                    def f():
                        if c == 0:
                            state["S"] = miscp.tile(
                                [128, TCH], f32, tag="misc", name=f"Sqk{tch}"
                            )
                        nc.tensor.matmul(
                            state["S"][:, :],
                            wqk_t[:, c, :],
                            xt[tch][:, c, :],
                            start=(c == 0),
                            stop=(c == N_CT - 1),
                            skip_group_check=True,
                        )
                    return f

                def qk_out():
                    S = state["S"]
                    qk_sb = qkp.tile(
                        [128, TCH], bf16, tag=f"qk_{tch}", name=f"qk_{tch}"
                    )
                    # ONE cast: q rows 0:64, k rows 64:128
                    nc.vector.tensor_copy(qk_sb[:], S[:])
                    dup = qkp.tile(
                        [128, TCH], bf16, tag=f"dup_{tch}", name=f"dup_{tch}"
                    )
                    # swapped-halves dup via DVE cross-partition casts (a DMA
                    # here would queue behind the whole input stream on the
                    # sync ring; SWDGE has ~4us latency). Chunk 0's own
                    # scores need the q-dup first; later chunks' jj=0 needs
                    # the k-dup first.
                    if tch == 0:
                        nc.vector.tensor_copy(dup[64:128, :], S[0:64, :])
                        nc.vector.tensor_copy(dup[0:64, :], S[64:128, :])
                    else:
                        nc.vector.tensor_copy(dup[0:64, :], S[64:128, :])
                        nc.vector.tensor_copy(dup[64:128, :], S[0:64, :])
                    qh[tch] = {0: qk_sb, 1: dup}
                    kh[tch] = {0: dup, 1: qk_sb}

                for c in range(N_CT):
                    steps.append(qk_mm(c))
                steps.append(qk_out)
                return steps

            def vpair_steps(ca, cb):
                """v projections for chunks ca/cb column-packed 2x on the PE
                into ONE [128,512] PSUM tile (ca -> partitions 0:64, cb ->
                64:128), one cast, then 8 combined transposes + v1 builds."""
                steps = []
                state = {}

                def v_mm(c):
                    def f():
                        if c == 0:
                            state["Pv"] = miscp.tile(
                                [128, TCH], f32, tag="misc", name=f"Pv{ca}"
                            )
                        nc.tensor.matmul(
                            state["Pv"][0:64, :],
                            wv_t[:, c, :],
                            xt[ca][:, c, :],
                            start=(c == 0),
                            stop=(c == N_CT - 1),
                            skip_group_check=True,
                        )
                        nc.tensor.matmul(
                            state["Pv"][64:128, :],
                            wv_t[:, c, :],
                            xt[cb][:, c, :],
                            start=(c == 0),
                            stop=(c == N_CT - 1),
                            skip_group_check=True,
                        )
                    return f

                def v_out():
                    vTab = vTp.tile(
                        [128, TCH], bf16, tag="vTab", name=f"vT{ca}{cb}"
                    )
                    nc.vector.tensor_copy(vTab[:], state["Pv"][:, :])
                    state["vTab"] = vTab

                def v1_build(i):
                    def f():
                        Pt = miscp.tile([128, 128], bf16, tag="misc", name=f"Pt{ca}_{i}")
                        nc.tensor.transpose(
                            Pt[:, :],
                            state["vTab"][:, 128 * i : 128 * (i + 1)],
                            idb_t[:, :],
                        )
                        # [128, 130]: cols 0:65 = [v|1] chunk ca tile i,
                        #             cols 65:130 = [v|1] chunk cb tile i
                        pairt = v1p.tile(
                            [128, 2, 65], bf16, tag=f"v1_{ca}_{i}", name=f"v1_{ca}_{i}"
                        )
                        nc.vector.tensor_copy(
                            pairt[:, :, 0:64],
                            Pt[:].rearrange("p (j m) -> p j m", j=2),
                        )
                        nc.gpsimd.memset(pairt[:, :, 64:65], 1.0)
                        v1[4 * ca + i] = pairt[:, 0, :]
                        v1[4 * cb + i] = pairt[:, 1, :]
                    return f

                for c in range(N_CT):
                    steps.append(v_mm(c))
                steps.append(v_out)
                for i in range(4):
                    steps.append(v1_build(i))
                return steps

            def emit_scores(tch, jp):
                """Scores matmuls for pair (jp, jp+1) -> PSUM tile."""
                S2 = Sp.tile([128, 2 * TCH], f32, tag="S", name=f"S{tch}_{jp}")
                los = {}
                for jj in range(2):
                    j = jp + jj
                    rel = j - 4 * tch
                    lo = 128 * max(0, rel)
                    los[jj] = lo
                    half = 1 if tch == 0 else jj
                    ksl = kh[j // 4][half][
                        64 * half : 64 * half + 64,
                        128 * (j % 4) : 128 * (j % 4 + 1),
                    ]
                    qsl = qh[tch][half][64 * half : 64 * half + 64, lo:TCH]
                    nc.tensor.matmul(
                        S2[:, TCH * jj + lo : TCH * (jj + 1)],
                        ksl,
                        qsl,
                        start=True,
                        stop=True,
                        skip_group_check=True,
                    )
                return S2, los

            def emit_exp(tch, jp, S2, los, block=None):
                """exp on ScalarE; trimmed to the valid window."""
                tag = "ex"
                if block is None:
                    ext = expp.tile([128, 2 * TCH], bf16, tag=tag, name=f"ex{tch}_{jp}")
                    a = los[0]
                    nc.scalar.activation(
                        ext[:, a:], S2[:, a:], EXP, scale=SCALE
                    )
                    return ext
                # per-block exp (tail): block 0 then block 1 into same tile
                ext, jj = block
                a = TCH * jj + los[jj]
                nc.scalar.activation(ext[:, a : TCH * (jj + 1)],
                                     S2[:, a : TCH * (jj + 1)], EXP, scale=SCALE)
                return ext

            def emit_pv_block(tch, j, jj, ext, lo, first, last):
                if j - 4 * tch >= 0:
                    a = TCH * jj + lo
                    nc.vector.tensor_mul(
                        ext[:, a : a + 128], ext[:, a : a + 128], mask_t[:]
                    )
                nc.tensor.matmul(
                    accs[tch][:, lo:TCH] if j > 0 else accs[tch][:, :],
                    v1[j],
                    ext[:, TCH * jj + lo : TCH * (jj + 1)],
                    start=first,
                    stop=last,
                    skip_group_check=True,
                )

            def emit_pv(tch, jp, ext, los):
                jmax = 4 * tch + 3
                for jj in range(2):
                    j = jp + jj
                    emit_pv_block(tch, j, jj, ext, los[jj], j == 0, j == jmax)

            epi_state = {}

            def emit_epilogue(tch, half):
                # ==== normalize + transpose + DMA out for chunk tch, half ====
                # half=0: cols [0:256) (final after the diagonal pair);
                # half=1: cols [256:512).
                csl = slice(256 * half, 256 * half + 256)
                oTh = epip.tile([65, 256], f32, tag="oTh", name=f"oT{tch}_{half}")
                nc.vector.tensor_copy(oTh[:, :], accs[tch][:, csl])
                Pe = miscp.tile([128, 2, 65], f32, tag="misc", name=f"Pe{tch}_{half}")
                for ii in range(2):
                    nc.tensor.transpose(
                        Pe[:, ii, :],
                        oTh[:, 128 * ii : 128 * (ii + 1)],
                        idf_t[0:65, 0:65],
                    )
                rec = epip.tile([128, 2], f32, tag="rec", name=f"rec{tch}_{half}")
                nc.vector.reciprocal(
                    rec[:].rearrange("p (i o) -> p i o", o=1), Pe[:, :, 64:65]
                )
                oth = epip.tile([128, 2, H], f32, tag="oth", name=f"ot{tch}_{half}")
                if FUSED_EPI:
                    import concourse.bass as _bass
                    rec3 = rec[:].rearrange("p (i o) -> p i o", o=1)
                    pe3 = Pe[:, :, 0:64]
                    rec_b, pe_b = _bass.broadcast_tensor_aps(rec3, pe3)
                    nc.vector.tensor_mul(oth[:, :, :], pe_b, rec_b)
                else:
                    for ii in range(2):
                        nc.vector.tensor_scalar_mul(
                            oth[:, ii, :], Pe[:, ii, 0:64], rec[:, ii : ii + 1]
                        )
                r0 = TCH * tch + 256 * half
                nc.sync.dma_start(
                    out=out_d[r0 : r0 + 256, :].rearrange("(i p) h -> p i h", i=2),
                    in_=oth[:, :, :],
                )

            # ---- the global pair stream ----
            slots = [
                (tch, jp) for tch in range(N_CHUNK) for jp in range(0, 4 * tch + 4, 2)
            ]
            slot_of = {p: k for k, p in enumerate(slots)}
            n_slots = len(slots)
            # paced projection queues with due slots
            for s in qk_steps(0):
                s()
            # chunk-1 qk projection eagerly behind chunk-0's
            for s in qk_steps(1):
                s()
            queues = {
                "qk1": [slot_of[(1, 0)], []],
                "v01": [slot_of[(0, 0)] + LAG, vpair_steps(0, 1)],
                "qk2": [slot_of[(2, 0)] - 1, qk_steps(2)],
                "qk3": [slot_of[(3, 0)] - 1, qk_steps(3)],
                "v23": [min(slot_of[(2, 8)] + LAG, n_slots), vpair_steps(2, 3)],
            }

            accs = {
                tch: accp.tile([65, TCH], f32, tag="acc", name=f"acc{tch}")
                for tch in range(N_CHUNK)
            }

            exts = {}

            def drain_due(k):
                for name, q in queues.items():
                    due, items = q
                    if not items or due - k > 8:  # not urgent yet
                        continue
                    left = max(1, due - k)
                    n = -(-len(items) // left)
                    for _ in range(n):
                        if items:
                            items.pop(0)()

            def force_drain(*names):
                for name in names:
                    items = queues[name][1]
                    while items:
                        items.pop(0)()

            def emit_pv_slot(kv):
                vt, vjp = slots[kv]
                if vjp >= 4 * vt:  # diagonal pair: needs own chunk's v1
                    force_drain("v01" if vt <= 1 else "v23")
                if (vt, vjp) == (3, 14):
                    # tail special-case: per-block exp/mask/PV interleave
                    S2, los = exts.pop(kv)
                    ext = expp.tile(
                        [128, 2 * TCH], bf16, tag="ex", name="ex3_14"
                    )
                    for jj in range(2):
                        emit_exp(3, 14, S2, los, block=(ext, jj))
                        emit_pv_block(3, 14 + jj, jj, ext, los[jj],
                                      False, 14 + jj == 15)
                else:
                    emit_pv(vt, vjp, *exts.pop(kv))
                if vjp == 4 * vt:
                    emit_epilogue(vt, half=0)
                elif vjp == 4 * vt + 2:
                    emit_epilogue(vt, half=1)

            for k, (tch, jp) in enumerate(slots):
                # hard guard: chunk's qk projection before its first scores
                if jp == 0 and tch >= 1:
                    force_drain(f"qk{tch}")
                S2, los = emit_scores(tch, jp)
                if (tch, jp) == (3, 14):
                    exts[k] = (S2, los)  # exp deferred to PV time (tail)
                else:
                    exts[k] = (emit_exp(tch, jp, S2, los), los)
                if k <= 1:
                    # keep the first score pairs adjacent; no pops yet
                    continue
                if k - LAG >= 0:
                    emit_pv_slot(k - LAG)
                drain_due(k)

            # trailing PVs + last epilogues
            for kv in range(n_slots - LAG, n_slots):
                emit_pv_slot(kv)

    nc.compile()
    return nc


def _get_nc():
    if "nc" not in _CACHE:
        _CACHE["nc"] = _build()
    return _CACHE["nc"]


def _tile_w(w):
    """[C, F] -> [128, N_CT*F] with c-tile-major column blocks."""
    Cdim, F = w.shape
    return np.ascontiguousarray(
        w.reshape(Cdim // 128, 128, F).transpose(1, 0, 2).reshape(128, -1)
    )


def _host_inputs(x, w_q, w_k, w_v):
    bf = ml_dtypes.bfloat16
    x = np.asarray(x, dtype=np.float32)
    wqk = np.concatenate(
        [np.asarray(w_q, np.float32), np.asarray(w_k, np.float32)], 1
    )
    wv = np.asarray(w_v, np.float32)
    wqk_tiled = _tile_w(wqk).astype(bf)
    wv_tiled = _tile_w(wv).astype(bf)
    # multiplicative causal mask for transposed-score diag blocks: keep s <= t
    mask01 = np.triu(np.ones((128, 128), np.float32)).astype(bf)
    idf = np.eye(128, dtype=np.float32)
    idb = np.eye(128, dtype=np.float32).astype(bf)
    in_maps = []
    for i in range(N_CORES):
        # x^T pre-tiled: [128, chunk, c-tile, t] flattened per partition
        xT = np.ascontiguousarray(x[i].T).astype(bf)  # [C, T]
        xT4 = xT.reshape(N_CT, 128, N_CHUNK, TCH)     # [c, p, chunk, t]
        xTt = xT4.transpose(1, 2, 0, 3).reshape(128, N_CHUNK, -1)  # [p, chunk, c*t]
        blob0 = np.ascontiguousarray(
            np.concatenate([wqk_tiled, xTt[:, 0, :], wv_tiled], axis=1)
        )
        xrest = np.ascontiguousarray(xTt[:, 1:, :].reshape(128, -1))
        in_maps.append(
            {
                "blob0": blob0,
                "xrest": xrest,
                "mask01": mask01,
                "idf": idf,
                "idb": idb,
            }
        )
    return in_maps


def run(x, w_q, w_k, w_v, trace=False, **trace_kwargs):
    from concourse.bass_utils import run_bass_kernel_spmd

    nc = _get_nc()
    in_maps = _host_inputs(x, w_q, w_k, w_v)
    res = run_bass_kernel_spmd(
        nc, in_maps, core_ids=list(range(N_CORES)), trace=trace, **trace_kwargs
    )
    out = np.stack([np.asarray(res.results[i]["out"]) for i in range(N_CORES)])
    return out.astype(np.float32), res


def kernel(x, w_q, w_k, w_v):
    out, _ = run(x, w_q, w_k, w_v, trace=False)
    return out


# revision 13
# speedup vs baseline: 1.1284x; 1.1284x over previous
"""Distributed Trainium2 kernel for a single attention head.

Problem: x:[8,2048,1024] f32, w_q/w_k/w_v:[1024,64] f32
  q,k,v = x@w ; scores = (q k^T)/sqrt(1024) causal-masked; out = softmax(scores)@v

Sharding: data-parallel over batch B=8 across the 8 NeuronCores (one batch
element per core, weights replicated, no collectives).

Per-core dataflow (T=2048, C=1024, H=64). v2 of the kernel; the structural
changes vs v1 are geared at PE-stream density and DVE offload:

  - blob0 = [wqk | x^T chunk0 | wv] so the first DMA cut carries exactly
    what the first projection matmuls need; xrest chunks ship in half-chunk
    cuts for finer arrival granularity. All input, dup and output DMAs ride
    the sync HWDGE queue in FIFO order (SWDGE latency ~4us is too slow for
    anything on the critical path); gpsimd SWDGE only carries mask/identity
    constants.
  - qk projection epilogue: ONE [128,512] DVE cast into qk_sb (q rows 0:64,
    k rows 64:128 as produced by the packed weights), then two sync-queue
    dup DMAs build dup_t (k at rows 0:64, q at rows 64:128). Scores for
    s-block j with moving q of chunk t: half-0 MM reads k from dup_t/q from
    qk_sb; half-1 MM reads k from qk_sb/q from dup_t. All pairs of chunks
    1-3 run row-tiled 2x concurrent; chunk-0 pairs run unpacked on half 1
    (q-dup only is on the critical path there).
  - v projections: both chunks of a pair column-packed into ONE [128,512]
    PSUM tile -> ONE cast to vTab (ca rows 0:64, cb rows 64:128) -> 8
    combined [128,128] PE transposes produce v1 data for BOTH chunks at
    once -> one strided DVE copy per transpose into a [128,130] v1-pair
    tile ([v|1] slots for both chunks); ones column memset on gpsimd.
  - exp on ScalarE trimmed to [lo0:1024] per pair; the last pair (3,14)
    is split per block so the tail epilogue starts ~0.6us earlier.
  - epilogue per half-chunk (cols 0:256 after the diagonal pair, 256:512
    after the next): PSUM->SBUF copy, 2 transposes into one [128,2,65]
    PSUM tile, ONE [128,2] reciprocal, broadcast multiply, DMA out.
  - causal: diagonal 128x128 blocks multiplied by a 0/1 mask on VectorE.
  - PV unchanged: out^T accumulated with lhsT = [v | 1] fused denominator,
    LAG slots behind scores.
"""

import os
import sys

import numpy as np

for p in ("/opt/trn_rl_repo",):
    if p not in sys.path and os.path.isdir(p):
        sys.path.insert(0, p)

import ml_dtypes  # noqa: E402

B, T, C, H = 8, 2048, 1024, 64
N_CORES = 8
TCH = 512                  # t-chunk
N_CHUNK = T // TCH         # 4
N_CT = C // 128            # 8 contraction tiles
SCALE = float(C) ** -0.5   # 1/32
N_WARM = 9                 # PE warm-up matmuls (N=512 each, ~0.43us cold)
LAG = 4                    # PV trails scores by this many pair slots
FUSED_EPI = True           # broadcast-mul epilogue (fallback: per-block)

_CACHE = {}


def _build():
    """Build + compile the SPMD Bass graph (same graph on all 8 cores)."""
    import concourse.bass as bass
    import concourse.mybir as mybir
    import concourse.tile as tile
    from concourse import bacc

    f32 = mybir.dt.float32
    bf16 = mybir.dt.bfloat16
    EXP = mybir.ActivationFunctionType.Exp

    nc = bacc.Bacc(
        "TRN2", target_bir_lowering=False, debug=False, num_devices=N_CORES
    )

    # blob0 = [wqk | x^T chunk0 | wv], x pre-tiled [c-tile][t]
    W_WQK = N_CT * 128
    W_X = N_CT * TCH
    W_WV = N_CT * H
    BLOB0_W = W_WQK + W_X + W_WV
    blob0_d = nc.dram_tensor("blob0", [128, BLOB0_W], bf16, kind="ExternalInput")
    xrest_d = nc.dram_tensor("xrest", [128, 3 * W_X], bf16, kind="ExternalInput")
    mask_d = nc.dram_tensor("mask01", [128, 128], bf16, kind="ExternalInput")
    idf_d = nc.dram_tensor("idf", [128, 128], f32, kind="ExternalInput")
    idb_d = nc.dram_tensor("idb", [128, 128], bf16, kind="ExternalInput")
    out_d = nc.dram_tensor("out", [T, H], f32, kind="ExternalOutput")

    with tile.TileContext(nc) as tc:
        with (
            tc.tile_pool(name="const", bufs=1) as constp,
            tc.tile_pool(name="xTp", bufs=1) as xTp,
            tc.tile_pool(name="qkp", bufs=1) as qkp,
            tc.tile_pool(name="vTp", bufs=2) as vTp,
            tc.tile_pool(name="v1p", bufs=1) as v1p,
            tc.tile_pool(name="exp", bufs=LAG + 2) as expp,
            tc.tile_pool(name="epi", bufs=2) as epip,
            tc.tile_pool(name="Sp", bufs=2, space="PSUM") as Sp,
            tc.tile_pool(name="accp", bufs=1, space="PSUM") as accp,
            tc.tile_pool(name="miscp", bufs=3, space="PSUM") as miscp,
        ):
            # ---- PE warm-up scratch (memset on gpsimd: earliest free engine)
            warm_sb = constp.tile([128, TCH], bf16, tag="warm_sb", name="warm_sb")
            nc.gpsimd.memset(warm_sb[:], 0.0)
            warm_act = constp.tile([128, 8], bf16, tag="warm_act", name="warm_act")
            warm_ps = miscp.tile([128, TCH], f32, tag="misc", name="warm_ps")
            for i in range(N_WARM):
                nc.tensor.matmul(
                    warm_ps[:, :],
                    warm_sb[:, 0:128],
                    warm_sb[:, :],
                    start=True,
                    stop=True,
                    skip_group_check=True,
                )

            # ---- input DMAs: all on the sync HWDGE queue, strict FIFO ----
            blob0_t = constp.tile([128, BLOB0_W], bf16, tag="blob0", name="blob0_t")
            # cuts: [wqk|t0-1], [t2-4], [t5-7], [wv]. Few, large DMAs: many
            # small cuts measurably degrade ring throughput (~20%) and the
            # per-engine completion spread (sem fires on the slowest of 16
            # engines) grows to ~3us.
            cuts = [
                0,
                W_WQK + 2 * TCH,
                W_WQK + 5 * TCH,
                W_WQK + W_X,
                BLOB0_W,
            ]
            for lo, hi in zip(cuts[:-1], cuts[1:]):
                nc.sync.dma_start(out=blob0_t[:, lo:hi], in_=blob0_d[:, lo:hi])
            wqk_t = blob0_t[:, 0:W_WQK].rearrange("p (n m) -> p n m", n=N_CT)
            xt = {
                0: blob0_t[:, W_WQK : W_WQK + W_X].rearrange(
                    "p (n m) -> p n m", n=N_CT
                )
            }
            wv_t = blob0_t[:, W_WQK + W_X :].rearrange("p (n m) -> p n m", n=N_CT)
            # xrest: chunk 1 in halves (its projection is tight behind the
            # front), chunks 2-3 whole
            xr_h = xrest_d[:].rearrange("p (t h n m) -> p t h n m", t=3, h=2, n=N_CT // 2)
            xr_f = xrest_d[:].rearrange("p (t n m) -> p t n m", t=3, n=N_CT)
            for t in (1, 2, 3):
                xx = xTp.tile([128, N_CT, TCH], bf16, tag=f"x{t}", name=f"x{t}")
                if t == 1:
                    xxh = xx[:].rearrange("p (h n) m -> p h n m", h=2)
                    for hh in range(2):
                        nc.sync.dma_start(out=xxh[:, hh], in_=xr_h[:, t - 1, hh])
                else:
                    nc.sync.dma_start(out=xx[:], in_=xr_f[:, t - 1])
                xt[t] = xx[:]

            # dummy exp: forces the ACT table-set load during the DMA phase
            nc.scalar.activation(warm_act[:], warm_sb[:, 0:8], EXP, scale=1.0)

            # ---- small constants on the gpsimd SWDGE queue ----
            mask_t = constp.tile([128, 128], bf16, tag="mask", name="mask_t")
            nc.gpsimd.dma_start(out=mask_t[:], in_=mask_d[:])
            idb_t = constp.tile([128, 128], bf16, tag="idb", name="idb_t")
            nc.gpsimd.dma_start(out=idb_t[:], in_=idb_d[:])
            idf_t = constp.tile([128, 128], f32, tag="idf", name="idf_t")
            nc.gpsimd.dma_start(out=idf_t[:], in_=idf_d[:])

            # per-chunk q/k tiles: qh[half] / kh[half] -> tile holding that
            # operand on that partition half (qk_sb or dup_t)
            qh = {}
            kh = {}
            v1 = {}    # [128, 65] bf16 AP per s-tile: [v | 1]

            # v1 pair tiles are persistent; pre-create them and set the ones
            # columns ONCE up front (a per-tile memset at build time chains
            # PV matmuls on the gpsimd queue, ~300ns each)
            v1pairs = {}
            for ca in (0, 2):
                for i in range(4):
                    pt = v1p.tile(
                        [128, 2, 65], bf16, tag=f"v1_{ca}_{i}", name=f"v1_{ca}_{i}"
                    )
                    nc.gpsimd.memset(pt[:, :, 64:65], 1.0)
                    v1pairs[(ca, i)] = pt

            def filler(n=1):
                """Warm-style matmuls to absorb known DMA-wait idle windows
                (keeps the HAM clock gate at 8/8 through the front)."""
                for _ in range(n):
                    nc.tensor.matmul(
                        warm_ps[:, :],
                        warm_sb[:, 0:128],
                        warm_sb[:, :],
                        start=True,
                        stop=True,
                        skip_group_check=True,
                    )

            def qk_steps(tch):
                """Emission thunks for chunk `tch`'s q/k projection."""
                steps = []
                state = {}

                def qk_mm(c):
                    def f():
                        if c == 0:
                            state["S"] = miscp.tile(
                                [128, TCH], f32, tag="misc", name=f"Sqk{tch}"
                            )
                        nc.tensor.matmul(
                            state["S"][:, :],
                            wqk_t[:, c, :],
                            xt[tch][:, c, :],
                            start=(c == 0),
                            stop=(c == N_CT - 1),
                            skip_group_check=True,
                        )
                    return f

                def qk_out():
                    S = state["S"]
                    qk_sb = qkp.tile(
                        [128, TCH], bf16, tag=f"qk_{tch}", name=f"qk_{tch}"
                    )
                    # ONE cast: q rows 0:64, k rows 64:128
                    nc.vector.tensor_copy(qk_sb[:], S[:])
                    dup = qkp.tile(
                        [128, TCH], bf16, tag=f"dup_{tch}", name=f"dup_{tch}"
                    )
                    # swapped-halves dup via DVE cross-partition casts (a DMA
                    # here would queue behind the whole input stream on the
                    # sync ring; SWDGE has ~4us latency). Chunk 0's own
                    # scores need the q-dup first; later chunks' jj=0 needs
                    # the k-dup first.
                    if tch == 0:
                        nc.vector.tensor_copy(dup[64:128, :], S[0:64, :])
                        nc.vector.tensor_copy(dup[0:64, :], S[64:128, :])
                    else:
                        nc.vector.tensor_copy(dup[0:64, :], S[64:128, :])
                        nc.vector.tensor_copy(dup[64:128, :], S[0:64, :])
                    qh[tch] = {0: qk_sb, 1: dup}
                    kh[tch] = {0: dup, 1: qk_sb}

                for c in range(N_CT):
                    steps.append(qk_mm(c))
                steps.append(qk_out)
                return steps

            def vpair_steps(ca, cb):
                """v projections for chunks ca/cb column-packed 2x on the PE
                into ONE [128,512] PSUM tile (ca -> partitions 0:64, cb ->
                64:128), one cast, then 8 combined transposes + v1 builds."""
                steps = []
                state = {}

                def v_mm(c):
                    def f():
                        if c == 0:
                            state["Pv"] = miscp.tile(
                                [128, TCH], f32, tag="misc", name=f"Pv{ca}"
                            )
                        nc.tensor.matmul(
                            state["Pv"][0:64, :],
                            wv_t[:, c, :],
                            xt[ca][:, c, :],
                            start=(c == 0),
                            stop=(c == N_CT - 1),
                            skip_group_check=True,
                        )
                        nc.tensor.matmul(
                            state["Pv"][64:128, :],
                            wv_t[:, c, :],
                            xt[cb][:, c, :],
                            start=(c == 0),
                            stop=(c == N_CT - 1),
                            skip_group_check=True,
                        )
                    return f

                def v_out():
                    vTab = vTp.tile(
                        [128, TCH], bf16, tag="vTab", name=f"vT{ca}{cb}"
                    )
                    nc.vector.tensor_copy(vTab[:], state["Pv"][:, :])
                    state["vTab"] = vTab

                def v1_build(i):
                    def f():
                        Pt = miscp.tile([128, 128], bf16, tag="misc", name=f"Pt{ca}_{i}")
                        nc.tensor.transpose(
                            Pt[:, :],
                            state["vTab"][:, 128 * i : 128 * (i + 1)],
                            idb_t[:, :],
                        )
                        # [128, 2, 65]: slot 0 = [v|1] chunk ca tile i,
                        #               slot 1 = [v|1] chunk cb tile i
                        pairt = v1pairs[(ca, i)]
                        nc.vector.tensor_copy(
                            pairt[:, :, 0:64],
                            Pt[:].rearrange("p (j m) -> p j m", j=2),
                        )
                        v1[4 * ca + i] = pairt[:, 0, :]
                        v1[4 * cb + i] = pairt[:, 1, :]
                    return f

                for c in range(N_CT):
                    steps.append(v_mm(c))
                steps.append(v_out)
                for i in range(4):
                    steps.append(v1_build(i))
                return steps

            def emit_scores(tch, jp):
                """Scores matmuls for pair (jp, jp+1) -> PSUM tile."""
                S2 = Sp.tile([128, 2 * TCH], f32, tag="S", name=f"S{tch}_{jp}")
                los = {}
                for jj in range(2):
                    j = jp + jj
                    rel = j - 4 * tch
                    lo = 128 * max(0, rel)
                    los[jj] = lo
                    half = 1 if tch == 0 else jj
                    ksl = kh[j // 4][half][
                        64 * half : 64 * half + 64,
                        128 * (j % 4) : 128 * (j % 4 + 1),
                    ]
                    qsl = qh[tch][half][64 * half : 64 * half + 64, lo:TCH]
                    nc.tensor.matmul(
                        S2[:, TCH * jj + lo : TCH * (jj + 1)],
                        ksl,
                        qsl,
                        start=True,
                        stop=True,
                        skip_group_check=True,
                    )
                return S2, los

            def emit_exp(tch, jp, S2, los, block=None):
                """exp on ScalarE; trimmed to the valid window."""
                tag = "ex"
                if block is None:
                    ext = expp.tile([128, 2 * TCH], bf16, tag=tag, name=f"ex{tch}_{jp}")
                    a = los[0]
                    nc.scalar.activation(
                        ext[:, a:], S2[:, a:], EXP, scale=SCALE
                    )
                    return ext
                # per-block exp (tail): block 0 then block 1 into same tile
                ext, jj = block
                a = TCH * jj + los[jj]
                nc.scalar.activation(ext[:, a : TCH * (jj + 1)],
                                     S2[:, a : TCH * (jj + 1)], EXP, scale=SCALE)
                return ext

            def emit_pv_block(tch, j, jj, ext, lo, first, last):
                if j - 4 * tch >= 0:
                    a = TCH * jj + lo
                    nc.vector.tensor_mul(
                        ext[:, a : a + 128], ext[:, a : a + 128], mask_t[:]
                    )
                nc.tensor.matmul(
                    accs[tch][:, lo:TCH] if j > 0 else accs[tch][:, :],
                    v1[j],
                    ext[:, TCH * jj + lo : TCH * (jj + 1)],
                    start=first,
                    stop=last,
                    skip_group_check=True,
                )

            def emit_pv(tch, jp, ext, los):
                jmax = 4 * tch + 3
                for jj in range(2):
                    j = jp + jj
                    emit_pv_block(tch, j, jj, ext, los[jj], j == 0, j == jmax)

            pending_epi = []

            def emit_epilogue(tch, half):
                # ==== normalize + transpose + DMA out for chunk tch, half ====
                # half=0: cols [0:256) (final after the diagonal pair);
                # half=1: cols [256:512).
                # The PSUM->SBUF copy is emitted now; the PE-side rest is
                # deferred one slot so the copy latency never blocks the
                # in-order PE queue.
                csl = slice(256 * half, 256 * half + 256)
                oTh = epip.tile([65, 256], f32, tag="oTh", name=f"oT{tch}_{half}")
                nc.vector.tensor_copy(oTh[:, :], accs[tch][:, csl])

                def rest():
                    Pe = miscp.tile([128, 2, 65], f32, tag="misc", name=f"Pe{tch}_{half}")
                    for ii in range(2):
                        nc.tensor.transpose(
                            Pe[:, ii, :],
                            oTh[:, 128 * ii : 128 * (ii + 1)],
                            idf_t[0:65, 0:65],
                        )
                    rec = epip.tile([128, 2], f32, tag="rec", name=f"rec{tch}_{half}")
                    nc.vector.reciprocal(
                        rec[:].rearrange("p (i o) -> p i o", o=1), Pe[:, :, 64:65]
                    )
                    oth = epip.tile([128, 2, H], f32, tag="oth", name=f"ot{tch}_{half}")
                    if FUSED_EPI:
                        import concourse.bass as _bass
                        rec3 = rec[:].rearrange("p (i o) -> p i o", o=1)
                        pe3 = Pe[:, :, 0:64]
                        rec_b, pe_b = _bass.broadcast_tensor_aps(rec3, pe3)
                        nc.vector.tensor_mul(oth[:, :, :], pe_b, rec_b)
                    else:
                        for ii in range(2):
                            nc.vector.tensor_scalar_mul(
                                oth[:, ii, :], Pe[:, ii, 0:64], rec[:, ii : ii + 1]
                            )
                    r0 = TCH * tch + 256 * half
                    nc.sync.dma_start(
                        out=out_d[r0 : r0 + 256, :].rearrange("(i p) h -> p i h", i=2),
                        in_=oth[:, :, :],
                    )

                pending_epi.append(rest)

            def flush_epi():
                while pending_epi:
                    pending_epi.pop(0)()

            # ---- the global pair stream ----
            slots = [
                (tch, jp) for tch in range(N_CHUNK) for jp in range(0, 4 * tch + 4, 2)
            ]
            slot_of = {p: k for k, p in enumerate(slots)}
            n_slots = len(slots)
            # paced projection queues with due slots. Chunk-0 qk runs first,
            # with fillers at the known blob0-cut wait points.
            qk0 = qk_steps(0)
            for ci, s in enumerate(qk0):
                if ci == 2:      # c2 waits cut1
                    filler(2)
                elif ci == 5:    # c5 waits cut2
                    filler(1)
                s()
            # chunk-1 qk projection eagerly behind chunk-0's
            for s in qk_steps(1):
                s()
            queues = {
                "qk1": [slot_of[(1, 0)], []],
                "v01": [slot_of[(0, 0)] + LAG, vpair_steps(0, 1)],
                "qk2": [slot_of[(2, 0)] - 1, qk_steps(2)],
                "qk3": [slot_of[(3, 0)] - 1, qk_steps(3)],
                "v23": [min(slot_of[(2, 8)] + LAG, n_slots), vpair_steps(2, 3)],
            }

            accs = {
                tch: accp.tile([65, TCH], f32, tag="acc", name=f"acc{tch}")
                for tch in range(N_CHUNK)
            }

            exts = {}

            def drain_due(k):
                for name, q in queues.items():
                    due, items = q
                    if not items or due - k > 8:  # not urgent yet
                        continue
                    left = max(1, due - k)
                    n = -(-len(items) // left)
                    for _ in range(n):
                        if items:
                            items.pop(0)()

            def force_drain(*names):
                for name in names:
                    items = queues[name][1]
                    while items:
                        items.pop(0)()

            def emit_pv_slot(kv):
                vt, vjp = slots[kv]
                if vjp >= 4 * vt:  # diagonal pair: needs own chunk's v1
                    force_drain("v01" if vt <= 1 else "v23")
                if (vt, vjp) == (3, 14):
                    # tail special-case: per-block exp/mask/PV interleave
                    S2, los = exts.pop(kv)
                    ext = expp.tile(
                        [128, 2 * TCH], bf16, tag="ex", name="ex3_14"
                    )
                    for jj in range(2):
                        emit_exp(3, 14, S2, los, block=(ext, jj))
                        emit_pv_block(3, 14 + jj, jj, ext, los[jj],
                                      False, 14 + jj == 15)
                else:
                    emit_pv(vt, vjp, *exts.pop(kv))
                if vjp == 4 * vt:
                    emit_epilogue(vt, half=0)
                elif vjp == 4 * vt + 2:
                    emit_epilogue(vt, half=1)

            for k, (tch, jp) in enumerate(slots):
                # hard guard: chunk's qk projection before its first scores
                if jp == 0 and tch >= 1:
                    force_drain(f"qk{tch}")
                if k == 0:
                    # fillers absorbing the cast+dup latency after the
                    # chunk-0 projection (PE would idle; a >3.4us idle
                    # re-throttles the HAM clock gate to half rate)
                    filler(5)
                elif k == 2:
                    # absorb the chunk-1 cast+dup latency
                    filler(3)
                S2, los = emit_scores(tch, jp)
                if (tch, jp) == (3, 14):
                    exts[k] = (S2, los)  # exp deferred to PV time (tail)
                else:
                    exts[k] = (emit_exp(tch, jp, S2, los), los)
                if k <= 1:
                    # keep the first score pairs adjacent; no pops yet
                    continue
                if k - LAG >= 0:
                    emit_pv_slot(k - LAG)
                flush_epi()
                drain_due(k)

            # trailing PVs + last epilogues
            for kv in range(n_slots - LAG, n_slots):
                emit_pv_slot(kv)
                flush_epi()

    nc.compile()
    return nc


def _get_nc():
    if "nc" not in _CACHE:
        _CACHE["nc"] = _build()
    return _CACHE["nc"]


def _tile_w(w):
    """[C, F] -> [128, N_CT*F] with c-tile-major column blocks."""
    Cdim, F = w.shape
    return np.ascontiguousarray(
        w.reshape(Cdim // 128, 128, F).transpose(1, 0, 2).reshape(128, -1)
    )


def _host_inputs(x, w_q, w_k, w_v):
    bf = ml_dtypes.bfloat16
    x = np.asarray(x, dtype=np.float32)
    wqk = np.concatenate(
        [np.asarray(w_q, np.float32), np.asarray(w_k, np.float32)], 1
    )
    wv = np.asarray(w_v, np.float32)
    wqk_tiled = _tile_w(wqk).astype(bf)
    wv_tiled = _tile_w(wv).astype(bf)
    # multiplicative causal mask for transposed-score diag blocks: keep s <= t
    mask01 = np.triu(np.ones((128, 128), np.float32)).astype(bf)
    idf = np.eye(128, dtype=np.float32)
    idb = np.eye(128, dtype=np.float32).astype(bf)
    in_maps = []
    for i in range(N_CORES):
        # x^T pre-tiled: [128, chunk, c-tile, t] flattened per partition
        xT = np.ascontiguousarray(x[i].T).astype(bf)  # [C, T]
        xT4 = xT.reshape(N_CT, 128, N_CHUNK, TCH)     # [c, p, chunk, t]
        xTt = xT4.transpose(1, 2, 0, 3).reshape(128, N_CHUNK, -1)  # [p, chunk, c*t]
        blob0 = np.ascontiguousarray(
            np.concatenate([wqk_tiled, xTt[:, 0, :], wv_tiled], axis=1)
        )
        xrest = np.ascontiguousarray(xTt[:, 1:, :].reshape(128, -1))
        in_maps.append(
            {
                "blob0": blob0,
                "xrest": xrest,
                "mask01": mask01,
                "idf": idf,
                "idb": idb,
            }
        )
    return in_maps


def run(x, w_q, w_k, w_v, trace=False, **trace_kwargs):
    from concourse.bass_utils import run_bass_kernel_spmd

    nc = _get_nc()
    in_maps = _host_inputs(x, w_q, w_k, w_v)
    res = run_bass_kernel_spmd(
        nc, in_maps, core_ids=list(range(N_CORES)), trace=trace, **trace_kwargs
    )
    out = np.stack([np.asarray(res.results[i]["out"]) for i in range(N_CORES)])
    return out.astype(np.float32), res


def kernel(x, w_q, w_k, w_v):
    out, _ = run(x, w_q, w_k, w_v, trace=False)
    return out


# revision 20
# speedup vs baseline: 1.1427x; 1.0127x over previous
"""Distributed Trainium2 kernel for a single attention head.

Problem: x:[8,2048,1024] f32, w_q/w_k/w_v:[1024,64] f32
  q,k,v = x@w ; scores = (q k^T)/sqrt(1024) causal-masked; out = softmax(scores)@v

Sharding: data-parallel over batch B=8 across the 8 NeuronCores (one batch
element per core, weights replicated, no collectives).

Per-core dataflow (T=2048, C=1024, H=64). v2 of the kernel; the structural
changes vs v1 are geared at PE-stream density and DVE offload:

  - blob0 = [wqk | x^T chunk0 | wv] so the first DMA cut carries exactly
    what the first projection matmuls need; xrest chunks ship in half-chunk
    cuts for finer arrival granularity. All input, dup and output DMAs ride
    the sync HWDGE queue in FIFO order (SWDGE latency ~4us is too slow for
    anything on the critical path); gpsimd SWDGE only carries mask/identity
    constants.
  - qk projection epilogue: ONE [128,512] DVE cast into qk_sb (q rows 0:64,
    k rows 64:128 as produced by the packed weights), then two sync-queue
    dup DMAs build dup_t (k at rows 0:64, q at rows 64:128). Scores for
    s-block j with moving q of chunk t: half-0 MM reads k from dup_t/q from
    qk_sb; half-1 MM reads k from qk_sb/q from dup_t. All pairs of chunks
    1-3 run row-tiled 2x concurrent; chunk-0 pairs run unpacked on half 1
    (q-dup only is on the critical path there).
  - v projections: both chunks of a pair column-packed into ONE [128,512]
    PSUM tile -> ONE cast to vTab (ca rows 0:64, cb rows 64:128) -> 8
    combined [128,128] PE transposes produce v1 data for BOTH chunks at
    once -> one strided DVE copy per transpose into a [128,130] v1-pair
    tile ([v|1] slots for both chunks); ones column memset on gpsimd.
  - exp on ScalarE trimmed to [lo0:1024] per pair; the last pair (3,14)
    is split per block so the tail epilogue starts ~0.6us earlier.
  - epilogue per half-chunk (cols 0:256 after the diagonal pair, 256:512
    after the next): PSUM->SBUF copy, 2 transposes into one [128,2,65]
    PSUM tile, ONE [128,2] reciprocal, broadcast multiply, DMA out.
  - causal: diagonal 128x128 blocks multiplied by a 0/1 mask on VectorE.
  - PV unchanged: out^T accumulated with lhsT = [v | 1] fused denominator,
    LAG slots behind scores.
"""

import os
import sys

import numpy as np

for p in ("/opt/trn_rl_repo",):
    if p not in sys.path and os.path.isdir(p):
        sys.path.insert(0, p)

import ml_dtypes  # noqa: E402

B, T, C, H = 8, 2048, 1024, 64
N_CORES = 8
TCH = 512                  # t-chunk
N_CHUNK = T // TCH         # 4
N_CT = C // 128            # 8 contraction tiles
SCALE = float(C) ** -0.5   # 1/32
N_WARM = 9                 # PE warm-up matmuls (N=512 each, ~0.43us cold)
LAG = 4                    # PV trails scores by this many pair slots
FUSED_EPI = True           # broadcast-mul epilogue (fallback: per-block)

_CACHE = {}


def _build():
    """Build + compile the SPMD Bass graph (same graph on all 8 cores)."""
    import concourse.bass as bass
    import concourse.mybir as mybir
    import concourse.tile as tile
    from concourse import bacc

    f32 = mybir.dt.float32
    bf16 = mybir.dt.bfloat16
    EXP = mybir.ActivationFunctionType.Exp

    nc = bacc.Bacc(
        "TRN2", target_bir_lowering=False, debug=False, num_devices=N_CORES
    )

    # blob0 = [wqk | x^T chunk0 | wv], x pre-tiled [c-tile][t]
    W_WQK = N_CT * 128
    W_X = N_CT * TCH
    W_WV = N_CT * H
    BLOB0_W = W_WQK + W_X + W_WV
    blob0_d = nc.dram_tensor("blob0", [128, BLOB0_W], bf16, kind="ExternalInput")
    xrest_d = nc.dram_tensor("xrest", [128, 3 * W_X], bf16, kind="ExternalInput")
    mask_d = nc.dram_tensor("mask01", [128, 128], bf16, kind="ExternalInput")
    idf_d = nc.dram_tensor("idf", [128, 128], f32, kind="ExternalInput")
    idb_d = nc.dram_tensor("idb", [128, 128], bf16, kind="ExternalInput")
    out_d = nc.dram_tensor("out", [T, H], f32, kind="ExternalOutput")

    with tile.TileContext(nc) as tc:
        with (
            tc.tile_pool(name="const", bufs=1) as constp,
            tc.tile_pool(name="xTp", bufs=1) as xTp,
            tc.tile_pool(name="qkp", bufs=1) as qkp,
            tc.tile_pool(name="vTp", bufs=2) as vTp,
            tc.tile_pool(name="v1p", bufs=1) as v1p,
            tc.tile_pool(name="exp", bufs=LAG + 2) as expp,
            tc.tile_pool(name="epi", bufs=2) as epip,
            tc.tile_pool(name="Sp", bufs=2, space="PSUM") as Sp,
            tc.tile_pool(name="accp", bufs=1, space="PSUM") as accp,
            tc.tile_pool(name="miscp", bufs=3, space="PSUM") as miscp,
        ):
            # ---- PE warm-up scratch (memset on gpsimd: earliest free engine)
            warm_sb = constp.tile([128, TCH], bf16, tag="warm_sb", name="warm_sb")
            nc.gpsimd.memset(warm_sb[:], 0.0)
            warm_act = constp.tile([128, 8], bf16, tag="warm_act", name="warm_act")
            warm_ps = miscp.tile([128, TCH], f32, tag="misc", name="warm_ps")
            for i in range(N_WARM):
                nc.tensor.matmul(
                    warm_ps[:, :],
                    warm_sb[:, 0:128],
                    warm_sb[:, :],
                    start=True,
                    stop=True,
                    skip_group_check=True,
                )

            # ---- input DMAs: all on the sync HWDGE queue, strict FIFO ----
            blob0_t = constp.tile([128, BLOB0_W], bf16, tag="blob0", name="blob0_t")
            # cuts: [wqk|t0-1], [t2-4], [t5-7], [wv]. Few, large DMAs: many
            # small cuts measurably degrade ring throughput (~20%) and the
            # per-engine completion spread (sem fires on the slowest of 16
            # engines) grows to ~3us.
            cuts = [
                0,
                W_WQK + 2 * TCH,
                W_WQK + 5 * TCH,
                W_WQK + W_X,
                BLOB0_W,
            ]
            for lo, hi in zip(cuts[:-2], cuts[1:-1]):
                nc.sync.dma_start(out=blob0_t[:, lo:hi], in_=blob0_d[:, lo:hi])
            wqk_t = blob0_t[:, 0:W_WQK].rearrange("p (n m) -> p n m", n=N_CT)
            xt = {
                0: blob0_t[:, W_WQK : W_WQK + W_X].rearrange(
                    "p (n m) -> p n m", n=N_CT
                )
            }
            wv_t = blob0_t[:, W_WQK + W_X :].rearrange("p (n m) -> p n m", n=N_CT)
            # xrest: chunk 1 in halves (its projection is tight behind the
            # front), chunks 2-3 whole; wv rides between xr1 and xr2 (first
            # needed by the v01 projections around the chunk-1 scores)
            xr_h = xrest_d[:].rearrange("p (t h n m) -> p t h n m", t=3, h=2, n=N_CT // 2)
            xr_f = xrest_d[:].rearrange("p (t n m) -> p t n m", t=3, n=N_CT)
            xt1 = xTp.tile([128, N_CT, TCH], bf16, tag="x1", name="x1")
            xt[1] = xt1[:]
            x1h = xt1[:].rearrange("p (h n) m -> p h n m", h=2)
            for hh in range(2):
                nc.sync.dma_start(out=x1h[:, hh], in_=xr_h[:, 0, hh])
            nc.sync.dma_start(
                out=blob0_t[:, cuts[-2] : cuts[-1]],
                in_=blob0_d[:, cuts[-2] : cuts[-1]],
            )
            for t in (2, 3):
                xx = xTp.tile([128, N_CT, TCH], bf16, tag=f"x{t}", name=f"x{t}")
                nc.sync.dma_start(out=xx[:], in_=xr_f[:, t - 1])
                xt[t] = xx[:]

            # dummy exp: forces the ACT table-set load during the DMA phase
            nc.scalar.activation(warm_act[:], warm_sb[:, 0:8], EXP, scale=1.0)

            # ---- small constants on the gpsimd SWDGE queue ----
            mask_t = constp.tile([128, 128], bf16, tag="mask", name="mask_t")
            nc.gpsimd.dma_start(out=mask_t[:], in_=mask_d[:])
            idb_t = constp.tile([128, 128], bf16, tag="idb", name="idb_t")
            nc.gpsimd.dma_start(out=idb_t[:], in_=idb_d[:])
            idf_t = constp.tile([128, 128], f32, tag="idf", name="idf_t")
            nc.gpsimd.dma_start(out=idf_t[:], in_=idf_d[:])

            # per-chunk q/k tiles: qh[half] / kh[half] -> tile holding that
            # operand on that partition half (qk_sb or dup_t)
            qh = {}
            kh = {}
            v1 = {}    # [128, 65] bf16 AP per s-tile: [v | 1]

            # v1 pair tiles are persistent; pre-create them and set the ones
            # columns ONCE up front (a per-tile memset at build time chains
            # PV matmuls on the gpsimd queue, ~300ns each)
            v1pairs = {}
            for ca in (0, 2):
                for i in range(4):
                    pt = v1p.tile(
                        [128, 2, 65], bf16, tag=f"v1_{ca}_{i}", name=f"v1_{ca}_{i}"
                    )
                    nc.gpsimd.memset(pt[:, :, 64:65], 1.0)
                    v1pairs[(ca, i)] = pt

            def filler(n=1):
                """Warm-style matmuls to absorb known DMA-wait idle windows
                (keeps the HAM clock gate at 8/8 through the front)."""
                for _ in range(n):
                    nc.tensor.matmul(
                        warm_ps[:, :],
                        warm_sb[:, 0:128],
                        warm_sb[:, :],
                        start=True,
                        stop=True,
                        skip_group_check=True,
                    )

            def qk_steps(tch):
                """Emission thunks for chunk `tch`'s q/k projection."""
                steps = []
                state = {}

                def qk_mm(c):
                    def f():
                        if c == 0:
                            state["S"] = miscp.tile(
                                [128, TCH], f32, tag="misc", name=f"Sqk{tch}"
                            )
                        nc.tensor.matmul(
                            state["S"][:, :],
                            wqk_t[:, c, :],
                            xt[tch][:, c, :],
                            start=(c == 0),
                            stop=(c == N_CT - 1),
                            skip_group_check=True,
                        )
                    return f

                def qk_out():
                    S = state["S"]
                    qk_sb = qkp.tile(
                        [128, TCH], bf16, tag=f"qk_{tch}", name=f"qk_{tch}"
                    )
                    qdup = qkp.tile(
                        [128, TCH], bf16, tag=f"qd_{tch}", name=f"qd_{tch}"
                    )
                    kdup = qkp.tile(
                        [128, TCH], bf16, tag=f"kd_{tch}", name=f"kd_{tch}"
                    )
                    # High priority: this chain gates the score stream and
                    # must not queue behind masks/epilogue work on the DVE.
                    # Separate qdup/kdup tiles: dependency tracking is
                    # tile-granular, one combined tile would make readers
                    # wait for both copies.
                    # A DMA here would queue behind the whole input stream
                    # on the sync ring; SWDGE has ~4us latency -> DVE casts.
                    with tc.high_priority():
                        # ONE cast: q rows 0:64, k rows 64:128
                        nc.vector.tensor_copy(qk_sb[:], S[:])
                        if tch == 0:
                            nc.vector.tensor_copy(qdup[64:128, :], S[0:64, :])
                            nc.vector.tensor_copy(kdup[0:64, :], S[64:128, :])
                        else:
                            nc.vector.tensor_copy(kdup[0:64, :], S[64:128, :])
                            nc.vector.tensor_copy(qdup[64:128, :], S[0:64, :])
                    qh[tch] = {0: qk_sb, 1: qdup}
                    kh[tch] = {0: kdup, 1: qk_sb}

                for c in range(N_CT):
                    steps.append(qk_mm(c))
                steps.append(qk_out)
                return steps

            def vpair_steps(ca, cb):
                """v projections for chunks ca/cb column-packed 2x on the PE
                into ONE [128,512] PSUM tile (ca -> partitions 0:64, cb ->
                64:128), one cast, then 8 combined transposes + v1 builds."""
                steps = []
                state = {}

                def v_mm(c):
                    def f():
                        if c == 0:
                            state["Pv"] = miscp.tile(
                                [128, TCH], f32, tag="misc", name=f"Pv{ca}"
                            )
                        nc.tensor.matmul(
                            state["Pv"][0:64, :],
                            wv_t[:, c, :],
                            xt[ca][:, c, :],
                            start=(c == 0),
                            stop=(c == N_CT - 1),
                            skip_group_check=True,
                        )
                        nc.tensor.matmul(
                            state["Pv"][64:128, :],
                            wv_t[:, c, :],
                            xt[cb][:, c, :],
                            start=(c == 0),
                            stop=(c == N_CT - 1),
                            skip_group_check=True,
                        )
                    return f

                def v_out():
                    vTab = vTp.tile(
                        [128, TCH], bf16, tag="vTab", name=f"vT{ca}{cb}"
                    )
                    with tc.high_priority():
                        nc.vector.tensor_copy(vTab[:], state["Pv"][:, :])
                    state["vTab"] = vTab

                def v1_build(i):
                    def f():
                        Pt = miscp.tile([128, 128], bf16, tag="misc", name=f"Pt{ca}_{i}")
                        nc.tensor.transpose(
                            Pt[:, :],
                            state["vTab"][:, 128 * i : 128 * (i + 1)],
                            idb_t[:, :],
                        )
                        # [128, 2, 65]: slot 0 = [v|1] chunk ca tile i,
                        #               slot 1 = [v|1] chunk cb tile i
                        pairt = v1pairs[(ca, i)]
                        nc.vector.tensor_copy(
                            pairt[:, :, 0:64],
                            Pt[:].rearrange("p (j m) -> p j m", j=2),
                        )
                        v1[4 * ca + i] = pairt[:, 0, :]
                        v1[4 * cb + i] = pairt[:, 1, :]
                    return f

                for c in range(N_CT):
                    steps.append(v_mm(c))
                steps.append(v_out)
                for i in range(4):
                    steps.append(v1_build(i))
                return steps

            def emit_scores(tch, jp):
                """Scores matmuls for pair (jp, jp+1) -> PSUM tile.

                chunk 0: unpacked on half 1 (k is native in qk_sb[64:128],
                  only the q-dup is on the critical path);
                chunk 1 pairs jp<=2: unpacked on half 0 (k of chunk 0 from
                  kdup, q native in qk_sb[0:64] -> no dup dependency);
                everything else: row-tiled 2x concurrent.
                """
                S2 = Sp.tile([128, 2 * TCH], f32, tag="S", name=f"S{tch}_{jp}")
                los = {}
                for jj in range(2):
                    j = jp + jj
                    rel = j - 4 * tch
                    lo = 128 * max(0, rel)
                    los[jj] = lo
                    if tch == 0:
                        half = 1
                    elif tch == 1 and jp <= 2:
                        half = 0
                    else:
                        half = jj
                    ksl = kh[j // 4][half][
                        64 * half : 64 * half + 64,
                        128 * (j % 4) : 128 * (j % 4 + 1),
                    ]
                    qsl = qh[tch][half][64 * half : 64 * half + 64, lo:TCH]
                    nc.tensor.matmul(
                        S2[:, TCH * jj + lo : TCH * (jj + 1)],
                        ksl,
                        qsl,
                        start=True,
                        stop=True,
                        skip_group_check=True,
                    )
                return S2, los

            def emit_exp(tch, jp, S2, los, block=None):
                """exp on ScalarE; trimmed to the valid window. Diagonal
                pairs get one ACT per block (skips the dead zone)."""
                tag = "ex"
                if block is None:
                    ext = expp.tile([128, 2 * TCH], bf16, tag=tag, name=f"ex{tch}_{jp}")
                    if los[0] or los[1]:
                        for jj in range(2):
                            a = TCH * jj + los[jj]
                            nc.scalar.activation(
                                ext[:, a : TCH * (jj + 1)],
                                S2[:, a : TCH * (jj + 1)],
                                EXP,
                                scale=SCALE,
                            )
                    else:
                        nc.scalar.activation(ext[:], S2[:], EXP, scale=SCALE)
                    return ext
                # per-block exp (tail): block 0 then block 1 into same tile
                ext, jj = block
                a = TCH * jj + los[jj]
                nc.scalar.activation(ext[:, a : TCH * (jj + 1)],
                                     S2[:, a : TCH * (jj + 1)], EXP, scale=SCALE)
                return ext

            def emit_pv_block(tch, j, jj, ext, lo, first, last):
                if j - 4 * tch >= 0:
                    a = TCH * jj + lo
                    nc.vector.tensor_mul(
                        ext[:, a : a + 128], ext[:, a : a + 128], mask_t[:]
                    )
                nc.tensor.matmul(
                    accs[tch][:, lo:TCH] if j > 0 else accs[tch][:, :],
                    v1[j],
                    ext[:, TCH * jj + lo : TCH * (jj + 1)],
                    start=first,
                    stop=last,
                    skip_group_check=True,
                )

            def emit_pv(tch, jp, ext, los):
                jmax = 4 * tch + 3
                for jj in range(2):
                    j = jp + jj
                    emit_pv_block(tch, j, jj, ext, los[jj], j == 0, j == jmax)

            pending_epi = []

            def emit_epilogue(tch, half):
                # ==== normalize + transpose + DMA out for chunk tch, half ====
                # half=0: cols [0:256) (final after the diagonal pair);
                # half=1: cols [256:512).
                # The PSUM->SBUF copy is emitted now; the PE-side rest is
                # deferred one slot so the copy latency never blocks the
                # in-order PE queue.
                csl = slice(256 * half, 256 * half + 256)
                oTh = epip.tile([65, 256], f32, tag="oTh", name=f"oT{tch}_{half}")
                nc.vector.tensor_copy(oTh[:, :], accs[tch][:, csl])

                def rest():
                    Pe = miscp.tile([128, 2, 65], f32, tag="misc", name=f"Pe{tch}_{half}")
                    for ii in range(2):
                        nc.tensor.transpose(
                            Pe[:, ii, :],
                            oTh[:, 128 * ii : 128 * (ii + 1)],
                            idf_t[0:65, 0:65],
                        )
                    rec = epip.tile([128, 2], f32, tag="rec", name=f"rec{tch}_{half}")
                    nc.vector.reciprocal(
                        rec[:].rearrange("p (i o) -> p i o", o=1), Pe[:, :, 64:65]
                    )
                    oth = epip.tile([128, 2, H], f32, tag="oth", name=f"ot{tch}_{half}")
                    if FUSED_EPI:
                        import concourse.bass as _bass
                        rec3 = rec[:].rearrange("p (i o) -> p i o", o=1)
                        pe3 = Pe[:, :, 0:64]
                        rec_b, pe_b = _bass.broadcast_tensor_aps(rec3, pe3)
                        nc.vector.tensor_mul(oth[:, :, :], pe_b, rec_b)
                    else:
                        for ii in range(2):
                            nc.vector.tensor_scalar_mul(
                                oth[:, ii, :], Pe[:, ii, 0:64], rec[:, ii : ii + 1]
                            )
                    r0 = TCH * tch + 256 * half
                    nc.sync.dma_start(
                        out=out_d[r0 : r0 + 256, :].rearrange("(i p) h -> p i h", i=2),
                        in_=oth[:, :, :],
                    )

                pending_epi.append(rest)

            def flush_epi():
                while pending_epi:
                    pending_epi.pop(0)()

            # ---- the global pair stream ----
            slots = [
                (tch, jp) for tch in range(N_CHUNK) for jp in range(0, 4 * tch + 4, 2)
            ]
            slot_of = {p: k for k, p in enumerate(slots)}
            n_slots = len(slots)
            # paced projection queues with due slots. Chunk-0 qk runs first,
            # with fillers at the known blob0-cut wait points.
            qk0 = qk_steps(0)
            for ci, s in enumerate(qk0):
                if ci == 2:      # c2 waits cut1
                    filler(2)
                elif ci == 5:    # c5 waits cut2
                    filler(1)
                s()
            # chunk-1 qk projection: first half eagerly behind chunk-0's
            # (xr1a arrives early); second half + cast deferred so an xr1b
            # wait can never sit ahead of the chunk-0 scores in the PE queue
            qk1 = qk_steps(1)
            for s in qk1[:4]:
                s()
            queues = {
                "qk1": [slot_of[(1, 0)], qk1[4:]],
                "v01": [slot_of[(0, 0)] + LAG, vpair_steps(0, 1)],
                "qk2": [slot_of[(2, 0)] - 1, qk_steps(2)],
                "qk3": [slot_of[(3, 0)] - 1, qk_steps(3)],
                "v23": [min(slot_of[(2, 8)] + LAG, n_slots), vpair_steps(2, 3)],
            }

            accs = {
                tch: accp.tile([65, TCH], f32, tag="acc", name=f"acc{tch}")
                for tch in range(N_CHUNK)
            }

            exts = {}

            def drain_due(k):
                for name, q in queues.items():
                    due, items = q
                    if not items or due - k > 8:  # not urgent yet
                        continue
                    left = max(1, due - k)
                    n = -(-len(items) // left)
                    for _ in range(n):
                        if items:
                            items.pop(0)()

            def force_drain(*names):
                for name in names:
                    items = queues[name][1]
                    while items:
                        items.pop(0)()

            def emit_pv_slot(kv):
                vt, vjp = slots[kv]
                if vjp >= 4 * vt:  # diagonal pair: needs own chunk's v1
                    force_drain("v01" if vt <= 1 else "v23")
                if (vt, vjp) == (3, 14):
                    # tail special-case: per-block exp/mask/PV interleave
                    S2, los = exts.pop(kv)
                    ext = expp.tile(
                        [128, 2 * TCH], bf16, tag="ex", name="ex3_14"
                    )
                    for jj in range(2):
                        emit_exp(3, 14, S2, los, block=(ext, jj))
                        emit_pv_block(3, 14 + jj, jj, ext, los[jj],
                                      False, 14 + jj == 15)
                else:
                    emit_pv(vt, vjp, *exts.pop(kv))
                if vjp == 4 * vt:
                    emit_epilogue(vt, half=0)
                elif vjp == 4 * vt + 2:
                    emit_epilogue(vt, half=1)

            for k, (tch, jp) in enumerate(slots):
                # hard guard: chunk's qk projection before its first scores
                if jp == 0 and tch >= 1:
                    force_drain(f"qk{tch}")
                if k == 0:
                    # fillers absorbing the cast+dup latency after the
                    # chunk-0 projection (PE would idle; a >3.4us idle
                    # re-throttles the HAM clock gate to half rate)
                    filler(2)
                elif k == 2:
                    # absorb the chunk-1 cast latency
                    filler(3)
                S2, los = emit_scores(tch, jp)
                if (tch, jp) == (3, 14):
                    exts[k] = (S2, los)  # exp deferred to PV time (tail)
                else:
                    exts[k] = (emit_exp(tch, jp, S2, los), los)
                if k <= 1:
                    # keep the first score pairs adjacent; no pops yet
                    continue
                if k - LAG >= 0:
                    emit_pv_slot(k - LAG)
                flush_epi()
                drain_due(k)

            # trailing PVs + last epilogues
            for kv in range(n_slots - LAG, n_slots):
                emit_pv_slot(kv)
                flush_epi()

    nc.compile()
    return nc


def _get_nc():
    if "nc" not in _CACHE:
        _CACHE["nc"] = _build()
    return _CACHE["nc"]


def _tile_w(w):
    """[C, F] -> [128, N_CT*F] with c-tile-major column blocks."""
    Cdim, F = w.shape
    return np.ascontiguousarray(
        w.reshape(Cdim // 128, 128, F).transpose(1, 0, 2).reshape(128, -1)
    )


def _host_inputs(x, w_q, w_k, w_v):
    bf = ml_dtypes.bfloat16
    x = np.asarray(x, dtype=np.float32)
    wqk = np.concatenate(
        [np.asarray(w_q, np.float32), np.asarray(w_k, np.float32)], 1
    )
    wv = np.asarray(w_v, np.float32)
    wqk_tiled = _tile_w(wqk).astype(bf)
    wv_tiled = _tile_w(wv).astype(bf)
    # multiplicative causal mask for transposed-score diag blocks: keep s <= t
    mask01 = np.triu(np.ones((128, 128), np.float32)).astype(bf)
    idf = np.eye(128, dtype=np.float32)
    idb = np.eye(128, dtype=np.float32).astype(bf)
    in_maps = []
    for i in range(N_CORES):
        # x^T pre-tiled: [128, chunk, c-tile, t] flattened per partition
        xT = np.ascontiguousarray(x[i].T).astype(bf)  # [C, T]
        xT4 = xT.reshape(N_CT, 128, N_CHUNK, TCH)     # [c, p, chunk, t]
        xTt = xT4.transpose(1, 2, 0, 3).reshape(128, N_CHUNK, -1)  # [p, chunk, c*t]
        blob0 = np.ascontiguousarray(
            np.concatenate([wqk_tiled, xTt[:, 0, :], wv_tiled], axis=1)
        )
        xrest = np.ascontiguousarray(xTt[:, 1:, :].reshape(128, -1))
        in_maps.append(
            {
                "blob0": blob0,
                "xrest": xrest,
                "mask01": mask01,
                "idf": idf,
                "idb": idb,
            }
        )
    return in_maps


def run(x, w_q, w_k, w_v, trace=False, **trace_kwargs):
    from concourse.bass_utils import run_bass_kernel_spmd

    nc = _get_nc()
    in_maps = _host_inputs(x, w_q, w_k, w_v)
    res = run_bass_kernel_spmd(
        nc, in_maps, core_ids=list(range(N_CORES)), trace=trace, **trace_kwargs
    )
    out = np.stack([np.asarray(res.results[i]["out"]) for i in range(N_CORES)])
    return out.astype(np.float32), res


def kernel(x, w_q, w_k, w_v):
    out, _ = run(x, w_q, w_k, w_v, trace=False)
    return out


# revision 25
# speedup vs baseline: 1.1492x; 1.0057x over previous
"""Distributed Trainium2 kernel for a single attention head.

Problem: x:[8,2048,1024] f32, w_q/w_k/w_v:[1024,64] f32
  q,k,v = x@w ; scores = (q k^T)/sqrt(1024) causal-masked; out = softmax(scores)@v

Sharding: data-parallel over batch B=8 across the 8 NeuronCores (one batch
element per core, weights replicated, no collectives).

Per-core dataflow (T=2048, C=1024, H=64). v2 of the kernel; the structural
changes vs v1 are geared at PE-stream density and DVE offload:

  - blob0 = [wqk | x^T chunk0 | wv] so the first DMA cut carries exactly
    what the first projection matmuls need; xrest chunks ship in half-chunk
    cuts for finer arrival granularity. All input, dup and output DMAs ride
    the sync HWDGE queue in FIFO order (SWDGE latency ~4us is too slow for
    anything on the critical path); gpsimd SWDGE only carries mask/identity
    constants.
  - qk projection epilogue: ONE [128,512] DVE cast into qk_sb (q rows 0:64,
    k rows 64:128 as produced by the packed weights), then two sync-queue
    dup DMAs build dup_t (k at rows 0:64, q at rows 64:128). Scores for
    s-block j with moving q of chunk t: half-0 MM reads k from dup_t/q from
    qk_sb; half-1 MM reads k from qk_sb/q from dup_t. All pairs of chunks
    1-3 run row-tiled 2x concurrent; chunk-0 pairs run unpacked on half 1
    (q-dup only is on the critical path there).
  - v projections: both chunks of a pair column-packed into ONE [128,512]
    PSUM tile -> ONE cast to vTab (ca rows 0:64, cb rows 64:128) -> 8
    combined [128,128] PE transposes produce v1 data for BOTH chunks at
    once -> one strided DVE copy per transpose into a [128,130] v1-pair
    tile ([v|1] slots for both chunks); ones column memset on gpsimd.
  - exp on ScalarE trimmed to [lo0:1024] per pair; the last pair (3,14)
    is split per block so the tail epilogue starts ~0.6us earlier.
  - epilogue per half-chunk (cols 0:256 after the diagonal pair, 256:512
    after the next): PSUM->SBUF copy, 2 transposes into one [128,2,65]
    PSUM tile, ONE [128,2] reciprocal, broadcast multiply, DMA out.
  - causal: diagonal 128x128 blocks multiplied by a 0/1 mask on VectorE.
  - PV unchanged: out^T accumulated with lhsT = [v | 1] fused denominator,
    LAG slots behind scores.
"""

import os
import sys

import numpy as np

for p in ("/opt/trn_rl_repo",):
    if p not in sys.path and os.path.isdir(p):
        sys.path.insert(0, p)

import ml_dtypes  # noqa: E402

B, T, C, H = 8, 2048, 1024, 64
N_CORES = 8
TCH = 512                  # t-chunk
N_CHUNK = T // TCH         # 4
N_CT = C // 128            # 8 contraction tiles
SCALE = float(C) ** -0.5   # 1/32
N_WARM = 8                 # PE warm-up matmuls (N=512 each, ~0.43us cold)
LAG = 4                    # PV trails scores by this many pair slots
FUSED_EPI = True           # broadcast-mul epilogue (fallback: per-block)

_CACHE = {}


def _build():
    """Build + compile the SPMD Bass graph (same graph on all 8 cores)."""
    import concourse.bass as bass
    import concourse.mybir as mybir
    import concourse.tile as tile
    from concourse import bacc

    f32 = mybir.dt.float32
    bf16 = mybir.dt.bfloat16
    EXP = mybir.ActivationFunctionType.Exp

    nc = bacc.Bacc(
        "TRN2", target_bir_lowering=False, debug=False, num_devices=N_CORES
    )

    # blob0 = [wqk | x^T chunk0 | wv], x pre-tiled [c-tile][t]
    W_WQK = N_CT * 128
    W_X = N_CT * TCH
    W_WV = N_CT * H
    BLOB0_W = W_WQK + W_X + W_WV
    blob0_d = nc.dram_tensor("blob0", [128, BLOB0_W], bf16, kind="ExternalInput")
    xrest_d = nc.dram_tensor("xrest", [128, 3 * W_X], bf16, kind="ExternalInput")
    mask_d = nc.dram_tensor("mask01", [128, 128], bf16, kind="ExternalInput")
    idf_d = nc.dram_tensor("idf", [128, 128], f32, kind="ExternalInput")
    idb_d = nc.dram_tensor("idb", [128, 128], bf16, kind="ExternalInput")
    out_d = nc.dram_tensor("out", [T, H], f32, kind="ExternalOutput")

    with tile.TileContext(nc) as tc:
        with (
            tc.tile_pool(name="const", bufs=1) as constp,
            tc.tile_pool(name="xTp", bufs=1) as xTp,
            tc.tile_pool(name="qkp", bufs=1) as qkp,
            tc.tile_pool(name="vTp", bufs=2) as vTp,
            tc.tile_pool(name="v1p", bufs=1) as v1p,
            tc.tile_pool(name="exp", bufs=LAG + 2) as expp,
            tc.tile_pool(name="epi", bufs=2) as epip,
            tc.tile_pool(name="Sp", bufs=2, space="PSUM") as Sp,
            tc.tile_pool(name="accp", bufs=1, space="PSUM") as accp,
            tc.tile_pool(name="miscp", bufs=3, space="PSUM") as miscp,
        ):
            # ---- PE warm-up scratch (memset on gpsimd: earliest free engine)
            warm_sb = constp.tile([128, TCH], bf16, tag="warm_sb", name="warm_sb")
            nc.gpsimd.memset(warm_sb[:], 0.0)
            warm_act = constp.tile([128, 8], bf16, tag="warm_act", name="warm_act")
            warm_ps = miscp.tile([128, TCH], f32, tag="misc", name="warm_ps")
            for i in range(N_WARM):
                nc.tensor.matmul(
                    warm_ps[:, :],
                    warm_sb[:, 0:128],
                    warm_sb[:, :],
                    start=True,
                    stop=True,
                    skip_group_check=True,
                )

            # ---- input DMAs: all on the sync HWDGE queue, strict FIFO ----
            blob0_t = constp.tile([128, BLOB0_W], bf16, tag="blob0", name="blob0_t")
            # cuts: [wqk|t0-3], [t4-7], [wv]. Few, large DMAs: many small
            # cuts measurably degrade ring throughput (~20%) and the
            # per-engine completion spread (sem fires on the slowest of 16
            # engines) grows to ~3us.
            cuts = [
                0,
                W_WQK + 4 * TCH,
                W_WQK + W_X,
                BLOB0_W,
            ]
            for lo, hi in zip(cuts[:-2], cuts[1:-1]):
                nc.sync.dma_start(out=blob0_t[:, lo:hi], in_=blob0_d[:, lo:hi])
            wqk_t = blob0_t[:, 0:W_WQK].rearrange("p (n m) -> p n m", n=N_CT)
            xt = {
                0: blob0_t[:, W_WQK : W_WQK + W_X].rearrange(
                    "p (n m) -> p n m", n=N_CT
                )
            }
            wv_t = blob0_t[:, W_WQK + W_X :].rearrange("p (n m) -> p n m", n=N_CT)
            # xrest: chunk 1 in halves (its projection is tight behind the
            # front), chunks 2-3 whole; wv rides between xr1 and xr2 (first
            # needed by the v01 projections around the chunk-1 scores)
            xr_h = xrest_d[:].rearrange("p (t h n m) -> p t h n m", t=3, h=2, n=N_CT // 2)
            xr_f = xrest_d[:].rearrange("p (t n m) -> p t n m", t=3, n=N_CT)
            xt1 = xTp.tile([128, N_CT, TCH], bf16, tag="x1", name="x1")
            xt[1] = xt1[:]
            x1h = xt1[:].rearrange("p (h n) m -> p h n m", h=2)
            for hh in range(2):
                nc.sync.dma_start(out=x1h[:, hh], in_=xr_h[:, 0, hh])
            nc.sync.dma_start(
                out=blob0_t[:, cuts[-2] : cuts[-1]],
                in_=blob0_d[:, cuts[-2] : cuts[-1]],
            )
            for t in (2, 3):
                xx = xTp.tile([128, N_CT, TCH], bf16, tag=f"x{t}", name=f"x{t}")
                nc.sync.dma_start(out=xx[:], in_=xr_f[:, t - 1])
                xt[t] = xx[:]

            # dummy exp: forces the ACT table-set load during the DMA phase
            nc.scalar.activation(warm_act[:], warm_sb[:, 0:8], EXP, scale=1.0)

            # ---- small constants on the gpsimd SWDGE queue ----
            mask_t = constp.tile([128, 128], bf16, tag="mask", name="mask_t")
            nc.gpsimd.dma_start(out=mask_t[:], in_=mask_d[:])
            idb_t = constp.tile([128, 128], bf16, tag="idb", name="idb_t")
            nc.gpsimd.dma_start(out=idb_t[:], in_=idb_d[:])
            idf_t = constp.tile([128, 128], f32, tag="idf", name="idf_t")
            nc.gpsimd.dma_start(out=idf_t[:], in_=idf_d[:])

            # per-chunk q/k tiles: qh[half] / kh[half] -> tile holding that
            # operand on that partition half (qk_sb or dup_t)
            qh = {}
            kh = {}
            v1 = {}    # [128, 65] bf16 AP per s-tile: [v | 1]

            # v1 pair tiles are persistent; pre-create them and set the ones
            # columns ONCE up front (a per-tile memset at build time chains
            # PV matmuls on the gpsimd queue, ~300ns each)
            v1pairs = {}
            for ca in (0, 2):
                for i in range(4):
                    pt = v1p.tile(
                        [128, 2, 65], bf16, tag=f"v1_{ca}_{i}", name=f"v1_{ca}_{i}"
                    )
                    nc.gpsimd.memset(pt[:, :, 64:65], 1.0)
                    v1pairs[(ca, i)] = pt

            def filler(n=1):
                """Warm-style matmuls to absorb known DMA-wait idle windows
                (keeps the HAM clock gate at 8/8 through the front)."""
                for _ in range(n):
                    nc.tensor.matmul(
                        warm_ps[:, :],
                        warm_sb[:, 0:128],
                        warm_sb[:, :],
                        start=True,
                        stop=True,
                        skip_group_check=True,
                    )

            def qk_steps(tch):
                """Emission thunks for chunk `tch`'s q/k projection."""
                steps = []
                state = {}

                def qk_mm(c):
                    def f():
                        if c == 0:
                            state["S"] = miscp.tile(
                                [128, TCH], f32, tag="misc", name=f"Sqk{tch}"
                            )
                        nc.tensor.matmul(
                            state["S"][:, :],
                            wqk_t[:, c, :],
                            xt[tch][:, c, :],
                            start=(c == 0),
                            stop=(c == N_CT - 1),
                            skip_group_check=True,
                        )
                    return f

                def qk_out():
                    S = state["S"]
                    qk_sb = qkp.tile(
                        [128, TCH], bf16, tag=f"qk_{tch}", name=f"qk_{tch}"
                    )
                    qdup = qkp.tile(
                        [128, TCH], bf16, tag=f"qd_{tch}", name=f"qd_{tch}"
                    )
                    kdup = qkp.tile(
                        [128, TCH], bf16, tag=f"kd_{tch}", name=f"kd_{tch}"
                    )
                    # High priority: this chain gates the score stream and
                    # must not queue behind masks/epilogue work on the DVE.
                    # Separate qdup/kdup tiles: dependency tracking is
                    # tile-granular, one combined tile would make readers
                    # wait for both copies.
                    # A DMA here would queue behind the whole input stream
                    # on the sync ring; SWDGE has ~4us latency -> DVE casts.
                    with tc.high_priority():
                        # ONE cast: q rows 0:64, k rows 64:128
                        nc.vector.tensor_copy(qk_sb[:], S[:])
                        if tch == 0:
                            nc.vector.tensor_copy(qdup[64:128, :], S[0:64, :])
                            nc.vector.tensor_copy(kdup[0:64, :], S[64:128, :])
                        else:
                            nc.vector.tensor_copy(kdup[0:64, :], S[64:128, :])
                            nc.vector.tensor_copy(qdup[64:128, :], S[0:64, :])
                    qh[tch] = {0: qk_sb, 1: qdup}
                    kh[tch] = {0: kdup, 1: qk_sb}

                for c in range(N_CT):
                    steps.append(qk_mm(c))
                steps.append(qk_out)
                return steps

            def vpair_steps(ca, cb):
                """v projections for chunks ca/cb column-packed 2x on the PE
                into ONE [128,512] PSUM tile (ca -> partitions 0:64, cb ->
                64:128), one cast, then 8 combined transposes + v1 builds."""
                steps = []
                state = {}

                def v_mm(c):
                    def f():
                        if c == 0:
                            state["Pv"] = miscp.tile(
                                [128, TCH], f32, tag="misc", name=f"Pv{ca}"
                            )
                        nc.tensor.matmul(
                            state["Pv"][0:64, :],
                            wv_t[:, c, :],
                            xt[ca][:, c, :],
                            start=(c == 0),
                            stop=(c == N_CT - 1),
                            skip_group_check=True,
                        )
                        nc.tensor.matmul(
                            state["Pv"][64:128, :],
                            wv_t[:, c, :],
                            xt[cb][:, c, :],
                            start=(c == 0),
                            stop=(c == N_CT - 1),
                            skip_group_check=True,
                        )
                    return f

                def v_out():
                    vTab = vTp.tile(
                        [128, TCH], bf16, tag="vTab", name=f"vT{ca}{cb}"
                    )
                    with tc.high_priority():
                        nc.vector.tensor_copy(vTab[:], state["Pv"][:, :])
                    state["vTab"] = vTab

                def v1_build(i):
                    def f():
                        Pt = miscp.tile([128, 128], bf16, tag="misc", name=f"Pt{ca}_{i}")
                        nc.tensor.transpose(
                            Pt[:, :],
                            state["vTab"][:, 128 * i : 128 * (i + 1)],
                            idb_t[:, :],
                        )
                        # [128, 2, 65]: slot 0 = [v|1] chunk ca tile i,
                        #               slot 1 = [v|1] chunk cb tile i
                        pairt = v1pairs[(ca, i)]
                        nc.vector.tensor_copy(
                            pairt[:, :, 0:64],
                            Pt[:].rearrange("p (j m) -> p j m", j=2),
                        )
                        v1[4 * ca + i] = pairt[:, 0, :]
                        v1[4 * cb + i] = pairt[:, 1, :]
                    return f

                for c in range(N_CT):
                    steps.append(v_mm(c))
                steps.append(v_out)
                for i in range(4):
                    steps.append(v1_build(i))
                return steps

            def emit_scores(tch, jp):
                """Scores matmuls for pair (jp, jp+1) -> PSUM tile.

                chunk 0: unpacked on half 1 (k is native in qk_sb[64:128],
                  only the q-dup is on the critical path);
                chunk 1 pairs jp<=2: unpacked on half 0 (k of chunk 0 from
                  kdup, q native in qk_sb[0:64] -> no dup dependency);
                everything else: row-tiled 2x concurrent.
                """
                S2 = Sp.tile([128, 2 * TCH], f32, tag="S", name=f"S{tch}_{jp}")
                los = {}
                for jj in range(2):
                    j = jp + jj
                    rel = j - 4 * tch
                    lo = 128 * max(0, rel)
                    los[jj] = lo
                    if tch == 0:
                        half = 1
                    elif tch == 1 and jp <= 2:
                        half = 0
                    else:
                        half = jj
                    ksl = kh[j // 4][half][
                        64 * half : 64 * half + 64,
                        128 * (j % 4) : 128 * (j % 4 + 1),
                    ]
                    qsl = qh[tch][half][64 * half : 64 * half + 64, lo:TCH]
                    nc.tensor.matmul(
                        S2[:, TCH * jj + lo : TCH * (jj + 1)],
                        ksl,
                        qsl,
                        start=True,
                        stop=True,
                        skip_group_check=True,
                    )
                return S2, los

            def emit_exp(tch, jp, S2, los, block=None):
                """exp on ScalarE; trimmed to the valid window. Diagonal
                pairs get one ACT per block (skips the dead zone)."""
                tag = "ex"
                if block is None:
                    ext = expp.tile([128, 2 * TCH], bf16, tag=tag, name=f"ex{tch}_{jp}")
                    if los[0] or los[1]:
                        for jj in range(2):
                            a = TCH * jj + los[jj]
                            nc.scalar.activation(
                                ext[:, a : TCH * (jj + 1)],
                                S2[:, a : TCH * (jj + 1)],
                                EXP,
                                scale=SCALE,
                            )
                    else:
                        nc.scalar.activation(ext[:], S2[:], EXP, scale=SCALE)
                    return ext
                # per-block exp (tail): block 0 then block 1 into same tile
                ext, jj = block
                a = TCH * jj + los[jj]
                nc.scalar.activation(ext[:, a : TCH * (jj + 1)],
                                     S2[:, a : TCH * (jj + 1)], EXP, scale=SCALE)
                return ext

            def emit_pv_block(tch, j, jj, ext, lo, first, last):
                if j - 4 * tch >= 0:
                    a = TCH * jj + lo
                    nc.vector.tensor_mul(
                        ext[:, a : a + 128], ext[:, a : a + 128], mask_t[:]
                    )
                nc.tensor.matmul(
                    accs[tch][:, lo:TCH] if j > 0 else accs[tch][:, :],
                    v1[j],
                    ext[:, TCH * jj + lo : TCH * (jj + 1)],
                    start=first,
                    stop=last,
                    skip_group_check=True,
                )

            def emit_pv(tch, jp, ext, los):
                jmax = 4 * tch + 3
                for jj in range(2):
                    j = jp + jj
                    emit_pv_block(tch, j, jj, ext, los[jj], j == 0, j == jmax)

            pending_epi = []

            def emit_epilogue(tch, half):
                # ==== normalize + transpose + DMA out for chunk tch, half ====
                # half=0: cols [0:256) (final after the diagonal pair);
                # half=1: cols [256:512).
                # The PSUM->SBUF copy is emitted now; the PE-side rest is
                # deferred one slot so the copy latency never blocks the
                # in-order PE queue.
                csl = slice(256 * half, 256 * half + 256)
                oTh = epip.tile([65, 256], f32, tag="oTh", name=f"oT{tch}_{half}")
                nc.vector.tensor_copy(oTh[:, :], accs[tch][:, csl])

                def rest():
                    Pe = miscp.tile([128, 2, 65], f32, tag="misc", name=f"Pe{tch}_{half}")
                    for ii in range(2):
                        nc.tensor.transpose(
                            Pe[:, ii, :],
                            oTh[:, 128 * ii : 128 * (ii + 1)],
                            idf_t[0:65, 0:65],
                        )
                    rec = epip.tile([128, 2], f32, tag="rec", name=f"rec{tch}_{half}")
                    nc.vector.reciprocal(
                        rec[:].rearrange("p (i o) -> p i o", o=1), Pe[:, :, 64:65]
                    )
                    oth = epip.tile([128, 2, H], f32, tag="oth", name=f"ot{tch}_{half}")
                    if FUSED_EPI:
                        import concourse.bass as _bass
                        rec3 = rec[:].rearrange("p (i o) -> p i o", o=1)
                        pe3 = Pe[:, :, 0:64]
                        rec_b, pe_b = _bass.broadcast_tensor_aps(rec3, pe3)
                        nc.vector.tensor_mul(oth[:, :, :], pe_b, rec_b)
                    else:
                        for ii in range(2):
                            nc.vector.tensor_scalar_mul(
                                oth[:, ii, :], Pe[:, ii, 0:64], rec[:, ii : ii + 1]
                            )
                    r0 = TCH * tch + 256 * half
                    nc.sync.dma_start(
                        out=out_d[r0 : r0 + 256, :].rearrange("(i p) h -> p i h", i=2),
                        in_=oth[:, :, :],
                    )

                pending_epi.append(rest)

            def flush_epi():
                while pending_epi:
                    pending_epi.pop(0)()

            # ---- the global pair stream ----
            slots = [
                (tch, jp) for tch in range(N_CHUNK) for jp in range(0, 4 * tch + 4, 2)
            ]
            slot_of = {p: k for k, p in enumerate(slots)}
            n_slots = len(slots)
            # paced projection queues with due slots. Chunk-0 qk runs first,
            # with fillers at the known blob0-cut wait points.
            qk0 = qk_steps(0)
            for ci, s in enumerate(qk0):
                if ci == 4:      # c4 waits the second blob0 cut
                    filler(2)
                s()
            # chunk-1 qk projection: first half eagerly behind chunk-0's
            # (xr1a arrives early); second half + cast deferred so an xr1b
            # wait can never sit ahead of the chunk-0 scores in the PE queue
            qk1 = qk_steps(1)
            for s in qk1[:4]:
                s()
            queues = {
                "qk1": [slot_of[(1, 0)], qk1[4:]],
                "v01": [slot_of[(0, 0)] + LAG, vpair_steps(0, 1)],
                "qk2": [slot_of[(2, 0)] - 1, qk_steps(2)],
                "qk3": [slot_of[(3, 0)] - 1, qk_steps(3)],
                "v23": [min(slot_of[(2, 8)] + LAG, n_slots), vpair_steps(2, 3)],
            }

            accs = {
                tch: accp.tile([65, TCH], f32, tag="acc", name=f"acc{tch}")
                for tch in range(N_CHUNK)
            }

            exts = {}

            def drain_due(k, names=None):
                for name, q in queues.items():
                    if names is not None and name not in names:
                        continue
                    due, items = q
                    if not items or due - k > 8:  # not urgent yet
                        continue
                    left = max(1, due - k)
                    n = -(-len(items) // left)
                    for _ in range(n):
                        if items:
                            items.pop(0)()

            def force_drain(*names):
                for name in names:
                    items = queues[name][1]
                    while items:
                        items.pop(0)()

            def emit_pv_slot(kv):
                vt, vjp = slots[kv]
                if vjp >= 4 * vt:  # diagonal pair: needs own chunk's v1
                    force_drain("v01" if vt <= 1 else "v23")
                if (vt, vjp) == (3, 14):
                    # tail special-case: per-block exp/mask/PV interleave
                    S2, los = exts.pop(kv)
                    ext = expp.tile(
                        [128, 2 * TCH], bf16, tag="ex", name="ex3_14"
                    )
                    for jj in range(2):
                        emit_exp(3, 14, S2, los, block=(ext, jj))
                        emit_pv_block(3, 14 + jj, jj, ext, los[jj],
                                      False, 14 + jj == 15)
                else:
                    emit_pv(vt, vjp, *exts.pop(kv))
                if vjp == 4 * vt:
                    emit_epilogue(vt, half=0)
                elif vjp == 4 * vt + 2:
                    emit_epilogue(vt, half=1)

            for k, (tch, jp) in enumerate(slots):
                # hard guard: chunk's qk projection before its first scores
                if jp == 0 and tch >= 1:
                    force_drain(f"qk{tch}")
                if k == 0:
                    # fillers absorbing the cast+dup latency after the
                    # chunk-0 projection (PE would idle; a >3.4us idle
                    # re-throttles the HAM clock gate to half rate)
                    filler(2)
                elif k == 2:
                    # absorb the chunk-1 cast latency
                    filler(3)
                S2, los = emit_scores(tch, jp)
                if (tch, jp) == (3, 14):
                    exts[k] = (S2, los)  # exp deferred to PV time (tail)
                else:
                    exts[k] = (emit_exp(tch, jp, S2, los), los)
                if k <= 1:
                    # keep the first score pairs adjacent; only pre-drain
                    # the v projections (their k=3-4 deadline otherwise
                    # piles ~2us of drains between early score pairs)
                    drain_due(k, names=("v01",))
                    continue
                if k - LAG >= 0:
                    emit_pv_slot(k - LAG)
                flush_epi()
                drain_due(k)

            # trailing PVs + last epilogues
            for kv in range(n_slots - LAG, n_slots):
                emit_pv_slot(kv)
                flush_epi()

    nc.compile()
    return nc


def _get_nc():
    if "nc" not in _CACHE:
        _CACHE["nc"] = _build()
    return _CACHE["nc"]


def _tile_w(w):
    """[C, F] -> [128, N_CT*F] with c-tile-major column blocks."""
    Cdim, F = w.shape
    return np.ascontiguousarray(
        w.reshape(Cdim // 128, 128, F).transpose(1, 0, 2).reshape(128, -1)
    )


def _host_inputs(x, w_q, w_k, w_v):
    bf = ml_dtypes.bfloat16
    x = np.asarray(x, dtype=np.float32)
    wqk = np.concatenate(
        [np.asarray(w_q, np.float32), np.asarray(w_k, np.float32)], 1
    )
    wv = np.asarray(w_v, np.float32)
    wqk_tiled = _tile_w(wqk).astype(bf)
    wv_tiled = _tile_w(wv).astype(bf)
    # multiplicative causal mask for transposed-score diag blocks: keep s <= t
    mask01 = np.triu(np.ones((128, 128), np.float32)).astype(bf)
    idf = np.eye(128, dtype=np.float32)
    idb = np.eye(128, dtype=np.float32).astype(bf)
    in_maps = []
    for i in range(N_CORES):
        # x^T pre-tiled: [128, chunk, c-tile, t] flattened per partition
        xT = np.ascontiguousarray(x[i].T).astype(bf)  # [C, T]
        xT4 = xT.reshape(N_CT, 128, N_CHUNK, TCH)     # [c, p, chunk, t]
        xTt = xT4.transpose(1, 2, 0, 3).reshape(128, N_CHUNK, -1)  # [p, chunk, c*t]
        blob0 = np.ascontiguousarray(
            np.concatenate([wqk_tiled, xTt[:, 0, :], wv_tiled], axis=1)
        )
        xrest = np.ascontiguousarray(xTt[:, 1:, :].reshape(128, -1))
        in_maps.append(
            {
                "blob0": blob0,
                "xrest": xrest,
                "mask01": mask01,
                "idf": idf,
                "idb": idb,
            }
        )
    return in_maps


def run(x, w_q, w_k, w_v, trace=False, **trace_kwargs):
    from concourse.bass_utils import run_bass_kernel_spmd

    nc = _get_nc()
    in_maps = _host_inputs(x, w_q, w_k, w_v)
    res = run_bass_kernel_spmd(
        nc, in_maps, core_ids=list(range(N_CORES)), trace=trace, **trace_kwargs
    )
    out = np.stack([np.asarray(res.results[i]["out"]) for i in range(N_CORES)])
    return out.astype(np.float32), res


def kernel(x, w_q, w_k, w_v):
    out, _ = run(x, w_q, w_k, w_v, trace=False)
    return out
